# revision 6
# baseline (speedup 1.0000x reference)
"""v1: fp8 DoubleRow phase C + bf16 FM/MLP (measured 150131 ns)."""

import sys

for _p in ("/opt/trn_rl_repo",):
    if _p not in sys.path:
        sys.path.append(_p)

import numpy as np
import ml_dtypes

B, F, D, O = 1024, 16384, 256, 4096
NCORES = 8
GROUPS = [[0, 1, 2, 3], [4, 5, 6, 7]]
Bc, Oc, Fj = B // 2, O // 4, F // 4
P = 128
KT_C = F // P
KT_A = Fj // P
G_C = 8
G_A = 4
WSCALE = 64.0
ZSCALE = 2.0
PSC = WSCALE * ZSCALE

_cache = {}


def _build(repeat=None, phases=("A", "B", "C"), cc_copy=False):
    import concourse.bass as bass
    import concourse.mybir as mybir
    import concourse.tile as tile
    from concourse import bacc

    f32 = mybir.dt.float32
    f32r = mybir.dt.float32r
    bf16 = mybir.dt.bfloat16
    fp8 = mybir.dt.float8e4
    DR = mybir.MatmulPerfMode.DoubleRow

    nc = bacc.Bacc("TRN2", target_bir_lowering=False)

    zT = nc.dram_tensor("zT", [F, Bc], fp8, kind="ExternalInput")
    xfm = nc.dram_tensor("xfm", [Fj, Bc], bf16, kind="ExternalInput")
    wT = nc.dram_tensor("wT", [F, Oc], fp8, kind="ExternalInput")
    embj = nc.dram_tensor("embj", [Fj, D], bf16, kind="ExternalInput")
    w1T = nc.dram_tensor("w1T", [D, D], bf16, kind="ExternalInput")
    w2T = nc.dram_tensor("w2T", [D, Oc], bf16, kind="ExternalInput")
    b1 = nc.dram_tensor("b1", [D], f32, kind="ExternalInput")
    b2 = nc.dram_tensor("b2", [Oc], f32, kind="ExternalInput")
    lb = nc.dram_tensor("lb", [Oc], f32, kind="ExternalInput")
    out_lin = nc.dram_tensor("out_lin", [Bc, Oc], f32, kind="ExternalOutput")
    out_int = nc.dram_tensor("out_int", [Bc, Oc], f32, kind="ExternalOutput")
    out_sum = nc.dram_tensor("out_sum", [Bc, Oc], f32, kind="ExternalOutput")

    zT_t = zT.rearrange("(kt p) b -> p kt b", p=P)
    xfm_t = xfm.rearrange("(kt p) b -> p kt b", p=P)
    wT_t = wT.rearrange("(kt p) o -> p kt o", p=P)
    embj_t = embj.rearrange("(kt p) d -> p kt d", p=P)
    w1T_t = w1T.rearrange("(kt p) d -> p kt d", p=P)
    w2T_t = w2T.rearrange("(kt p) o -> p kt o", p=P)
    out_lin_t = out_lin.rearrange("(mt p) o -> p mt o", p=P)
    out_int_t = out_int.rearrange("(mt p) o -> p mt o", p=P)
    out_sum_t = out_sum.rearrange("(mt p) o -> p mt o", p=P)

    with tile.TileContext(nc) as tc:
        with (
            tc.tile_pool(name="const", bufs=1) as const,
            tc.tile_pool(name="dram", bufs=1, space="DRAM") as dram,
        ):
            ones_f = const.tile([1, P], f32, name="ones_f")
            nc.vector.memset(ones_f[:], 1.0)
            ones = const.tile([1, P], f32r, name="ones")
            nc.vector.tensor_copy(ones[:], ones_f[:])
            b1t = const.tile([P, 2], f32, name="b1t")
            nc.sync.dma_start(out=b1t[:], in_=b1.rearrange("(t p) -> p t", p=P))
            b2row = const.tile([1, Oc], f32r, name="b2row")
            nc.gpsimd.dma_start(out=b2row[:], in_=b2.rearrange("(a o) -> a o", a=1))
            lbrow = const.tile([1, Oc], f32r, name="lbrow")
            nc.gpsimd.dma_start(out=lbrow[:], in_=lb.rearrange("(a o) -> a o", a=1))
            keep = const
            intsb = keep.tile([P, 4, Oc], f32, name="intsb")
            lin = keep.tile([P, 4, Oc], f32, name="lin")
            ccin = dram.tile([4 * P, Bc], f32, name="ccin")
            ccout = dram.tile([4 * P, Bc], f32, name="ccout")
            ccin_t = ccin.rearrange("(t p) b -> p t b", p=P)
            ccout_t = ccout.rearrange("(t p) b -> p t b", p=P)

            def phase_A():
                with (
                    tc.tile_pool(name="emb_pool", bufs=1) as emb_pool,
                    tc.tile_pool(name="xf_pool", bufs=3) as xf_pool,
                    tc.tile_pool(name="x2_pool", bufs=2) as x2_pool,
                    tc.tile_pool(name="psA", bufs=1, space="PSUM") as psA,
                    tc.tile_pool(name="evA", bufs=1) as evA,
                ):
                    embt = emb_pool.tile([P, KT_A, D], bf16, name="embt")
                    nc.gpsimd.dma_start(out=embt[:], in_=embj_t[:])
                    emb2t = emb_pool.tile([P, KT_A, D], bf16, name="emb2t")
                    nc.vector.tensor_mul(emb2t[:], embt[:], embt[:])
                    se = [psA.tile([P, Bc], f32, tag=f"se{mt}", name=f"se{mt}") for mt in range(2)]
                    sq = [psA.tile([P, Bc], f32, tag=f"sq{mt}", name=f"sq{mt}") for mt in range(2)]
                    for kg in range(KT_A // G_A):
                        xf = xf_pool.tile([P, G_A, Bc], bf16, tag="xf", name="xf")
                        nc.gpsimd.dma_start(out=xf[:], in_=xfm_t[:, kg * G_A:(kg + 1) * G_A, :])
                        x2 = x2_pool.tile([P, G_A, Bc], bf16, tag="x2", name="x2")
                        nc.vector.tensor_mul(x2[:], xf[:], xf[:])
                        for g in range(G_A):
                            kt = kg * G_A + g
                            st, sp = kt == 0, kt == KT_A - 1
                            for mt in range(2):
                                nc.tensor.matmul(se[mt][:], embt[:, kt, mt * P:(mt + 1) * P],
                                                 xf[:, g], start=st, stop=sp)
                                nc.tensor.matmul(sq[mt][:], emb2t[:, kt, mt * P:(mt + 1) * P],
                                                 x2[:, g], start=st, stop=sp)
                    ev = evA.tile([P, 4, Bc], f32, name="ev")
                    for mt in range(2):
                        nc.vector.tensor_copy(ev[:, mt, :], se[mt][:])
                        nc.vector.tensor_copy(ev[:, 2 + mt, :], sq[mt][:])
                    nc.gpsimd.dma_start(out=ccin_t[:], in_=ev[:])
                if cc_copy:
                    nc.sync.dma_start(out=ccout[:], in_=ccin[:])
                else:
                    nc.gpsimd.collective_compute(
                        "AllReduce", mybir.AluOpType.add, replica_groups=GROUPS,
                        ins=[ccin.opt()], outs=[ccout.opt()],
                    )

            def phase_B():
                with (
                    tc.tile_pool(name="mlp", bufs=1) as mlp,
                    tc.tile_pool(name="psB", bufs=2, space="PSUM") as psB,
                ):
                    red = mlp.tile([P, 4, Bc], f32, name="red")
                    nc.sync.dma_start(out=red[:], in_=ccout_t[:])
                    w1s = mlp.tile([P, 2, D], bf16, name="w1s")
                    nc.gpsimd.dma_start(out=w1s[:], in_=w1T_t[:])
                    w2s = mlp.tile([P, 2, Oc], bf16, name="w2s")
                    nc.gpsimd.dma_start(out=w2s[:], in_=w2T_t[:])
                    t1 = mlp.tile([P, 2, Bc], f32, name="t1")
                    se_r, sq_r = red[:, 0:2, :], red[:, 2:4, :]
                    nc.vector.tensor_mul(t1[:], se_r, se_r)
                    nc.vector.tensor_sub(t1[:], t1[:], sq_r)
                    iv = mlp.tile([P, 2, Bc], bf16, name="iv")
                    nc.vector.tensor_scalar_mul(iv[:], t1[:], 0.5)
                    hsb = mlp.tile([P, 2, Bc], bf16, name="hsb")
                    for mt in range(2):
                        hp = psB.tile([P, Bc], f32, tag="hp", name="hp")
                        for kt in range(2):
                            nc.tensor.matmul(hp[:], w1s[:, kt, mt * P:(mt + 1) * P],
                                             iv[:, kt, :], start=(kt == 0), stop=(kt == 1))
                        nc.scalar.activation(hsb[:, mt, :], hp[:],
                                             mybir.ActivationFunctionType.Relu,
                                             bias=b1t[:, mt:mt + 1])
                    for mb in range(4):
                        for no in range(2):
                            pi = psB.tile([P, 512], f32, tag="pi", name="pi")
                            nc.tensor.matmul(pi[:], ones[:],
                                             b2row[:, no * 512:(no + 1) * 512],
                                             start=True, stop=False)
                            for kt in range(2):
                                nc.tensor.matmul(pi[:], hsb[:, kt, mb * P:(mb + 1) * P],
                                                 w2s[:, kt, no * 512:(no + 1) * 512],
                                                 start=False, stop=(kt == 1))
                            nc.vector.tensor_copy(intsb[:, mb, no * 512:(no + 1) * 512], pi[:])
                    nc.sync.dma_start(out=out_int_t[:], in_=intsb[:])

            def phase_C():
                with (
                    tc.tile_pool(name="xt_pool", bufs=3) as xt_pool,
                    tc.tile_pool(name="wt_pool", bufs=3) as wt_pool,
                    tc.tile_pool(name="psC", bufs=1, space="PSUM") as psC,
                ):
                    ps = [[psC.tile([P, 512], f32, tag=f"ps{m}{n}", name=f"ps{m}{n}")
                           for n in range(2)] for m in range(4)]
                    for m in range(4):
                        for n in range(2):
                            nc.tensor.matmul(ps[m][n][:], ones[:],
                                             lbrow[:, n * 512:(n + 1) * 512],
                                             start=True, stop=False)
                    for kg in range(KT_C // G_C):
                        xt = xt_pool.tile([P, G_C, Bc], fp8, tag="xt", name="xt")
                        wt = wt_pool.tile([P, G_C, Oc], fp8, tag="wt", name="wt")
                        nc.gpsimd.dma_start(out=xt[:], in_=zT_t[:, kg * G_C:(kg + 1) * G_C, :])
                        nc.sync.dma_start(out=wt[:], in_=wT_t[:, kg * G_C:(kg + 1) * G_C, :])
                        for g2 in range(G_C // 2):
                            k2 = kg * (G_C // 2) + g2
                            gs = slice(2 * g2, 2 * g2 + 2)
                            for m in range(4):
                                lhsT = xt[:, gs, m * P:(m + 1) * P]
                                for n in range(2):
                                    nc.tensor.matmul(ps[m][n][:], lhsT,
                                                     wt[:, gs, n * 512:(n + 1) * 512],
                                                     start=False, stop=(k2 == KT_C // 2 - 1),
                                                     perf_mode=DR)
                    for m in range(4):
                        for n in range(2):
                            nc.vector.tensor_scalar_mul(
                                lin[:, m, n * 512:(n + 1) * 512], ps[m][n][:], 1.0 / PSC)
                    nc.sync.dma_start(out=out_lin_t[:], in_=lin[:])

            def body():
                if "A" in phases:
                    phase_A()
                if "C" in phases:
                    phase_C()
                if "B" in phases:
                    phase_B()
                if "B" in phases and "C" in phases:
                    nc.vector.tensor_add(intsb[:], intsb[:], lin[:])
                    nc.sync.dma_start(out=out_sum_t[:], in_=intsb[:])

            if repeat is None:
                body()
            else:
                import concourse.mybir as _mb
                with tc.For_i(0, repeat, 1, hint_engines=(_mb.EngineType.PE,)) as _i:
                    body()
    nc.compile()
    return nc


def _prep_inputs(sae_features, emb, lin_w, lin_b, w1, b1, w2, b2):
    e4 = ml_dtypes.float8_e4m3
    bf = ml_dtypes.bfloat16
    x = np.asarray(sae_features, dtype=np.float32)
    emb = np.asarray(emb, dtype=np.float32)
    lin_w = np.asarray(lin_w, dtype=np.float32)
    w1T = np.ascontiguousarray(np.asarray(w1, np.float32).T).astype(bf)
    w2 = np.asarray(w2, dtype=np.float32)
    b1 = np.asarray(b1, np.float32)
    b2 = np.asarray(b2, np.float32)
    lin_b = np.asarray(lin_b, np.float32)

    z8 = ((x - 0.5) * ZSCALE).astype(e4)
    w8 = (lin_w * WSCALE).astype(e4)
    xb = x.astype(bf)
    bias_c = PSC * (lin_b + 0.5 * lin_w.sum(axis=1, dtype=np.float64).astype(np.float32))

    zT_half = [np.ascontiguousarray(z8[g * Bc:(g + 1) * Bc, :].T) for g in range(2)]
    xbT_half = [np.ascontiguousarray(xb[g * Bc:(g + 1) * Bc, :].T) for g in range(2)]
    wT_q = [np.ascontiguousarray(w8[j * Oc:(j + 1) * Oc, :].T) for j in range(4)]
    w2T_q = [np.ascontiguousarray(w2[j * Oc:(j + 1) * Oc, :].T).astype(bf) for j in range(4)]
    in_maps = []
    for c in range(NCORES):
        g, j = c // 4, c % 4
        in_maps.append({
            "zT": zT_half[g],
            "xfm": np.ascontiguousarray(xbT_half[g][j * Fj:(j + 1) * Fj, :]),
            "wT": wT_q[j],
            "embj": np.ascontiguousarray(emb[j * Fj:(j + 1) * Fj, :]).astype(bf),
            "w1T": w1T,
            "w2T": w2T_q[j],
            "b1": b1,
            "b2": np.ascontiguousarray(b2[j * Oc:(j + 1) * Oc]),
            "lb": np.ascontiguousarray(bias_c[j * Oc:(j + 1) * Oc]),
        })
    return in_maps


def _gather(results):
    outs = {}
    for key in ("out_sum", "out_lin", "out_int"):
        full = np.empty((B, O), dtype=np.float32)
        for c in range(NCORES):
            g, j = c // 4, c % 4
            full[g * Bc:(g + 1) * Bc, j * Oc:(j + 1) * Oc] = results[c][key]
        outs[key] = full
    return outs["out_sum"], outs["out_lin"], outs["out_int"]


def kernel(sae_features, emb, lin_w, lin_b, w1, b1, w2, b2):
    from concourse.bass_utils import run_bass_kernel_spmd

    if "nc" not in _cache:
        _cache["nc"] = _build()
    nc = _cache["nc"]
    in_maps = _prep_inputs(sae_features, emb, lin_w, lin_b, w1, b1, w2, b2)
    try:
        res = run_bass_kernel_spmd(nc, in_maps, list(range(NCORES)))
    except Exception:
        import time as _time
        _time.sleep(5)
        res = run_bass_kernel_spmd(nc, in_maps, list(range(NCORES)))
    return _gather(res.results)


# revision 10
# speedup vs baseline: 1.1435x; 1.1435x over previous
"""Neural Factorization Machine — Trainium2 Bass kernel, 8 NeuronCores.

Math (see reference):
    sum_emb = x @ emb; sum_sq = (x*x) @ (emb*emb)
    iv      = 0.5 * (sum_emb^2 - sum_sq)               [B, D]
    h       = relu(iv @ w1.T + b1)                     [B, D]
    inter   = h @ w2.T + b2                            [B, O]
    linear  = x @ lin_w.T + lin_b                      [B, O]
    out     = linear + inter
Returns (out, linear, inter) like the reference.

Sharding (8 cores, core c -> g = c//4 batch half, j = c%4 O-quarter):
  - big linear: rows g*512:(g+1)*512, cols j*1024:(j+1)*1024  (2x4 grid)
  - FM partial sums: core j of each half reduces F-slice j*4096:(j+1)*4096,
    AllReduce(add) over groups [[0..3],[4..7]] completes sum_emb/sum_sq
  - MLP replicated per half (tiny); second layer column-sharded by j.

Precision (validated against the 2e-2 gate; measured rel err 1.16e-2):
  - big linear in fp8 e4m3 with DoubleRow (2 k-planes per PE pass -> fp8
    peak 157 TF/s): host centers x as z = 2*(x-0.5) and scales w by 64;
    the exact rank-1 term 0.5*colsum(w)+lin_b enters via a ones-row
    PSUM-init matmul (x128 scale) and the PSUM is descaled by 1/128 at
    evacuation. Centering halves both quantization error terms.
  - FM sum_sq in fp8 DoubleRow (positive-sum matmul: quantization noise
    averages out; host ships x^2 and 1024*emb^2 in e4m3, PSUM/1024 at evac).
  - FM sum_emb + MLP in bf16 (signed cancellation / short contraction make
    fp8 fail the gate there; simulated and measured).
  - Accumulation always fp32 PSUM; out_int/out_sum shipped bf16 (host
    upcasts), out_lin fp32.

Schedule notes (all measured, not assumed):
  - phase C is a pure PE-bound DoubleRow stream (a DMA-free variant of the
    same MM stream times identically), ~215 ns per 512-col pass when warm.
  - phase order A -> C -> B: the AllReduce latency hides behind C's stream;
    B consumes the reduced sums after C.
  - early C-bias PSUM inits (banks 4-7, disjoint from A's) run before A to
    fill the initial DMA wait and warm the PE clock gate.
  - engine queues: sync carries input loads (embt/wt/red/w1s/w2s), gpsimd
    carries xf/x2/xt, scalar carries PSUM evacuation (Copy with 1/PSC
    scale), output DMAs and ccin, so no output DMA ever blocks a load.
  - the b2==0 case (true for this problem) skips the 8 ones@b2row PSUM
    inits; picked per-input at build time, so the timed program matches.
  - G_C=8 (4 DoubleRow pairs per DMA group) measured faster than 16.
"""

import sys

for _p in ("/opt/trn_rl_repo",):
    if _p not in sys.path:
        sys.path.append(_p)

import numpy as np
import ml_dtypes

B, F, D, O = 1024, 16384, 256, 4096
NCORES = 8
GROUPS = [[0, 1, 2, 3], [4, 5, 6, 7]]
Bc, Oc, Fj = B // 2, O // 4, F // 4   # per-core: 512 batch rows, 1024 O cols, 4096 F slice
P = 128
KT_C = F // P      # 128 k-tiles, big linear (paired 2x for DoubleRow)
KT_A = Fj // P     # 32 k-tiles, FM partials
G_C = 8            # k-tiles per DMA group, phase C (4 DoubleRow pairs)
G_A = 4            # k-tiles per DMA group, phase A
WSCALE = 64.0      # host scale on lin_w before fp8 quantization
ZSCALE = 2.0       # host scale on (x - 0.5) before fp8 quantization
PSC = WSCALE * ZSCALE  # PSUM holds PSC * linear; descaled at evacuation
E2SCALE = 1024.0   # host scale on emb^2 before fp8 quantization

_cache = {}


def _build(repeat=None, phases=("A", "B", "C"), cc_copy=False, b2_zero=False):
    """Emit the SPMD program. repeat=None -> single pass (the real kernel).
    repeat=R wraps the phase body in a hardware loop for timing.
    cc_copy=True replaces the AllReduce with a local DRAM copy (timing-only:
    collectives inside a hardware loop desync the mesh)."""
    import concourse.bass as bass
    import concourse.mybir as mybir
    import concourse.tile as tile
    from concourse import bacc

    f32 = mybir.dt.float32
    f32r = mybir.dt.float32r
    bf16 = mybir.dt.bfloat16
    fp8 = mybir.dt.float8e4
    DR = mybir.MatmulPerfMode.DoubleRow

    nc = bacc.Bacc("TRN2", target_bir_lowering=False)

    zT = nc.dram_tensor("zT", [F, Bc], fp8, kind="ExternalInput")
    xfm = nc.dram_tensor("xfm", [Fj, Bc], bf16, kind="ExternalInput")
    x2fm = nc.dram_tensor("x2fm", [Fj, Bc], fp8, kind="ExternalInput")
    wT = nc.dram_tensor("wT", [F, Oc], fp8, kind="ExternalInput")
    embj = nc.dram_tensor("embj", [Fj, D], bf16, kind="ExternalInput")
    embj2 = nc.dram_tensor("embj2", [Fj, D], fp8, kind="ExternalInput")
    w1T = nc.dram_tensor("w1T", [D, D], bf16, kind="ExternalInput")
    w2T = nc.dram_tensor("w2T", [D, Oc], bf16, kind="ExternalInput")
    b1 = nc.dram_tensor("b1", [D], f32, kind="ExternalInput")
    b2 = nc.dram_tensor("b2", [Oc], f32, kind="ExternalInput")
    lb = nc.dram_tensor("lb", [Oc], f32, kind="ExternalInput")  # 128*(lin_b+0.5*colsum w)
    out_lin = nc.dram_tensor("out_lin", [Bc, Oc], f32, kind="ExternalOutput")
    out_int = nc.dram_tensor("out_int", [Bc, Oc], bf16, kind="ExternalOutput")
    out_sum = nc.dram_tensor("out_sum", [Bc, Oc], bf16, kind="ExternalOutput")

    zT_t = zT.rearrange("(kt p) b -> p kt b", p=P)
    xfm_t = xfm.rearrange("(kt p) b -> p kt b", p=P)
    x2fm_t = x2fm.rearrange("(kt p) b -> p kt b", p=P)
    wT_t = wT.rearrange("(kt p) o -> p kt o", p=P)
    embj_t = embj.rearrange("(kt p) d -> p kt d", p=P)
    embj2_t = embj2.rearrange("(kt p) d -> p kt d", p=P)
    w1T_t = w1T.rearrange("(kt p) d -> p kt d", p=P)
    w2T_t = w2T.rearrange("(kt p) o -> p kt o", p=P)
    out_lin_t = out_lin.rearrange("(mt p) o -> p mt o", p=P)
    out_int_t = out_int.rearrange("(mt p) o -> p mt o", p=P)
    out_sum_t = out_sum.rearrange("(mt p) o -> p mt o", p=P)

    with tile.TileContext(nc) as tc:
        with (
            tc.tile_pool(name="const", bufs=1) as const,
            tc.tile_pool(name="dram", bufs=1, space="DRAM") as dram,
        ):
            ones_f = const.tile([1, P], f32, name="ones_f")
            nc.vector.memset(ones_f[:], 1.0)
            ones = const.tile([1, P], f32r, name="ones")
            nc.vector.tensor_copy(ones[:], ones_f[:])
            b1t = const.tile([P, 2], f32, name="b1t")
            nc.sync.dma_start(out=b1t[:], in_=b1.rearrange("(t p) -> p t", p=P))
            b2row = const.tile([1, Oc], f32r, name="b2row")
            nc.gpsimd.dma_start(out=b2row[:], in_=b2.rearrange("(a o) -> a o", a=1))
            lbrow = const.tile([1, Oc], f32r, name="lbrow")
            nc.gpsimd.dma_start(out=lbrow[:], in_=lb.rearrange("(a o) -> a o", a=1))
            # persistent SBUF intermediates across phases
            keep = const
            intsb = keep.tile([P, 4, Oc], bf16, name="intsb")
            sums = keep.tile([P, 4, Oc], bf16, name="sums")
            lin = keep.tile([P, 4, Oc], f32, name="lin")
            ccin = dram.tile([4 * P, Bc], f32, name="ccin")
            ccout = dram.tile([4 * P, Bc], f32, name="ccout")
            ccin_t = ccin.rearrange("(t p) b -> p t b", p=P)
            ccout_t = ccout.rearrange("(t p) b -> p t b", p=P)

            def phase_A():
                """FM partial sums over this core's F-slice -> ccin, AllReduce.
                sum_emb (se) in bf16; sum_sq (sq) in fp8 DoubleRow."""
                with (
                    tc.tile_pool(name="emb_pool", bufs=1) as emb_pool,
                    tc.tile_pool(name="xf_pool", bufs=3) as xf_pool,
                    tc.tile_pool(name="x2_pool", bufs=3) as x2_pool,
                    tc.tile_pool(name="psA", bufs=1, space="PSUM") as psA,
                    tc.tile_pool(name="evA", bufs=1) as evA,
                ):
                    embt = emb_pool.tile([P, KT_A, D], bf16, name="embt")
                    for ch in range(4):
                        nc.sync.dma_start(out=embt[:, ch * 8:(ch + 1) * 8, :],
                                          in_=embj_t[:, ch * 8:(ch + 1) * 8, :])
                    embt2 = emb_pool.tile([P, KT_A, D], fp8, name="embt2")
                    for ch in range(2):
                        nc.sync.dma_start(out=embt2[:, ch * 16:(ch + 1) * 16, :],
                                          in_=embj2_t[:, ch * 16:(ch + 1) * 16, :])
                    se = [psA.tile([P, Bc], f32, tag=f"se{mt}", name=f"se{mt}") for mt in range(2)]
                    sq = [psA.tile([P, Bc], f32, tag=f"sq{mt}", name=f"sq{mt}") for mt in range(2)]
                    for kg in range(KT_A // G_A):
                        xf = xf_pool.tile([P, G_A, Bc], bf16, tag="xf", name="xf")
                        x2 = x2_pool.tile([P, G_A, Bc], fp8, tag="x2", name="x2")
                        if kg == 0:
                            nc.gpsimd.dma_start(out=xf[:, 0:2, :], in_=xfm_t[:, 0:2, :])
                            nc.gpsimd.dma_start(out=x2[:, 0:2, :], in_=x2fm_t[:, 0:2, :])
                            nc.gpsimd.dma_start(out=xf[:, 2:4, :], in_=xfm_t[:, 2:4, :])
                            nc.gpsimd.dma_start(out=x2[:, 2:4, :], in_=x2fm_t[:, 2:4, :])
                        else:
                            nc.gpsimd.dma_start(out=xf[:], in_=xfm_t[:, kg * G_A:(kg + 1) * G_A, :])
                            nc.gpsimd.dma_start(out=x2[:], in_=x2fm_t[:, kg * G_A:(kg + 1) * G_A, :])
                        for g in range(G_A):
                            kt = kg * G_A + g
                            st, sp = kt == 0, kt == KT_A - 1
                            for mt in range(2):
                                nc.tensor.matmul(se[mt][:], embt[:, kt, mt * P:(mt + 1) * P],
                                                 xf[:, g], start=st, stop=sp)
                        for g2 in range(G_A // 2):
                            kt2 = kg * (G_A // 2) + g2
                            st, sp = kt2 == 0, kt2 == KT_A // 2 - 1
                            gs = slice(2 * g2, 2 * g2 + 2)
                            for mt in range(2):
                                nc.tensor.matmul(sq[mt][:],
                                                 embt2[:, 2 * kt2:2 * kt2 + 2, mt * P:(mt + 1) * P],
                                                 x2[:, gs, :], start=st, stop=sp,
                                                 perf_mode=DR)
                    ev = evA.tile([P, 4, Bc], f32, name="ev")
                    for mt in range(2):
                        nc.vector.tensor_copy(ev[:, mt, :], se[mt][:])
                        nc.vector.tensor_scalar_mul(ev[:, 2 + mt, :], sq[mt][:], 1.0 / E2SCALE)
                    nc.scalar.dma_start(out=ccin_t[:], in_=ev[:])
                if cc_copy:
                    nc.sync.dma_start(out=ccout[:], in_=ccin[:])
                else:
                    nc.gpsimd.collective_compute(
                        "AllReduce", mybir.AluOpType.add, replica_groups=GROUPS,
                        ins=[ccin.opt()], outs=[ccout.opt()],
                    )

            def phase_B():
                """iv -> h -> interaction_out (+b2) -> intsb/sums, out_int/out_sum."""
                fuse_sum = "C" in phases
                with (
                    tc.tile_pool(name="mlp", bufs=1) as mlp,
                    tc.tile_pool(name="psB", bufs=2, space="PSUM") as psB,
                ):
                    red = mlp.tile([P, 4, Bc], f32, name="red")
                    nc.sync.dma_start(out=red[:], in_=ccout_t[:])
                    w1s = mlp.tile([P, 2, D], bf16, name="w1s")
                    nc.sync.dma_start(out=w1s[:], in_=w1T_t[:])
                    w2s = mlp.tile([P, 2, Oc], bf16, name="w2s")
                    nc.sync.dma_start(out=w2s[:], in_=w2T_t[:])
                    t1 = mlp.tile([P, 2, Bc], f32, name="t1")
                    se_r, sq_r = red[:, 0:2, :], red[:, 2:4, :]
                    nc.vector.tensor_mul(t1[:], se_r, se_r)
                    nc.vector.tensor_sub(t1[:], t1[:], sq_r)
                    iv = mlp.tile([P, 2, Bc], bf16, name="iv")
                    nc.vector.tensor_scalar_mul(iv[:], t1[:], 0.5)
                    hsb = mlp.tile([P, 2, Bc], bf16, name="hsb")
                    for mt in range(2):
                        hp = psB.tile([P, Bc], f32, tag="hp", name="hp")
                        for kt in range(2):
                            nc.tensor.matmul(hp[:], w1s[:, kt, mt * P:(mt + 1) * P],
                                             iv[:, kt, :], start=(kt == 0), stop=(kt == 1))
                        nc.scalar.activation(hsb[:, mt, :], hp[:],
                                             mybir.ActivationFunctionType.Relu,
                                             bias=b1t[:, mt:mt + 1])
                    for mb in range(4):
                        for no in range(2):
                            pi = psB.tile([P, 512], f32, tag="pi", name="pi")
                            if not b2_zero:
                                nc.tensor.matmul(pi[:], ones[:],
                                                 b2row[:, no * 512:(no + 1) * 512],
                                                 start=True, stop=False)
                            for kt in range(2):
                                nc.tensor.matmul(pi[:], hsb[:, kt, mb * P:(mb + 1) * P],
                                                 w2s[:, kt, no * 512:(no + 1) * 512],
                                                 start=(b2_zero and kt == 0), stop=(kt == 1))
                            sl = slice(no * 512, (no + 1) * 512)
                            nc.vector.tensor_copy(intsb[:, mb, sl], pi[:])
                            if fuse_sum:
                                nc.vector.tensor_add(sums[:, mb, sl], pi[:], lin[:, mb, sl])
                        nc.scalar.dma_start(out=out_int_t[:, mb, :], in_=intsb[:, mb, :])
                        if fuse_sum:
                            nc.scalar.dma_start(out=out_sum_t[:, mb, :], in_=sums[:, mb, :])

            def phase_C(ps23):
                """linear = (zT.T @ wT)/PSC + bias-row ; fp8 DoubleRow."""
                with (
                    tc.tile_pool(name="xt_pool", bufs=3) as xt_pool,
                    tc.tile_pool(name="wt_pool", bufs=3) as wt_pool,
                    tc.tile_pool(name="psC", bufs=1, space="PSUM") as psC,
                ):
                    ps01 = [[psC.tile([P, 512], f32, tag=f"ps{m}{n}", name=f"ps{m}{n}")
                             for n in range(2)] for m in range(2)]
                    ps = ps01 + ps23
                    for m in (0, 1):
                        for n in range(2):
                            nc.tensor.matmul(ps[m][n][:], ones[:],
                                             lbrow[:, n * 512:(n + 1) * 512],
                                             start=True, stop=False)
                    for kg in range(KT_C // G_C):
                        xt = xt_pool.tile([P, G_C, Bc], fp8, tag="xt", name="xt")
                        wt = wt_pool.tile([P, G_C, Oc], fp8, tag="wt", name="wt")
                        nc.gpsimd.dma_start(out=xt[:], in_=zT_t[:, kg * G_C:(kg + 1) * G_C, :])
                        nc.sync.dma_start(out=wt[:], in_=wT_t[:, kg * G_C:(kg + 1) * G_C, :])
                        for g2 in range(G_C // 2):
                            k2 = kg * (G_C // 2) + g2
                            gs = slice(2 * g2, 2 * g2 + 2)
                            for m in range(4):
                                lhsT = xt[:, gs, m * P:(m + 1) * P]
                                for n in range(2):
                                    nc.tensor.matmul(ps[m][n][:], lhsT,
                                                     wt[:, gs, n * 512:(n + 1) * 512],
                                                     start=False, stop=(k2 == KT_C // 2 - 1),
                                                     perf_mode=DR)
                    for m in range(4):
                        for n in range(2):
                            nc.scalar.activation(
                                lin[:, m, n * 512:(n + 1) * 512], ps[m][n][:],
                                mybir.ActivationFunctionType.Copy, scale=1.0 / PSC)
                        nc.scalar.dma_start(out=out_lin_t[:, m, :], in_=lin[:, m, :])

            def body():
                # Order: early C-bias inits (PSUM banks disjoint from A's) fill
                # the initial DMA wait; A issues the AllReduce, whose latency
                # hides behind phase C's long matmul stream; B (which consumes
                # the reduced sums) runs after C; the epilogue fuses outputs.
                with tc.tile_pool(name="psCe", bufs=1, space="PSUM") as psCe:
                    ps23 = [[psCe.tile([P, 512], f32, tag=f"ps{m}{n}e", name=f"ps{m}{n}e")
                             for n in range(2)] for m in (2, 3)]
                    if "C" in phases:
                        for mi in range(2):
                            for n in range(2):
                                nc.tensor.matmul(ps23[mi][n][:], ones[:],
                                                 lbrow[:, n * 512:(n + 1) * 512],
                                                 start=True, stop=False)
                    if "A" in phases:
                        phase_A()
                    if "C" in phases:
                        phase_C(ps23)
                    if "B" in phases:
                        phase_B()

            if repeat is None:
                body()
            else:
                import concourse.mybir as _mb
                with tc.For_i(0, repeat, 1, hint_engines=(_mb.EngineType.PE,)) as _i:
                    body()
    nc.compile()
    return nc


def _prep_inputs(sae_features, emb, lin_w, lin_b, w1, b1, w2, b2):
    """Host-side shard + transpose + quantize. Returns in_maps for cores 0..7."""
    e4 = ml_dtypes.float8_e4m3
    bf = ml_dtypes.bfloat16
    x = np.asarray(sae_features, dtype=np.float32)
    emb = np.asarray(emb, dtype=np.float32)
    lin_w = np.asarray(lin_w, dtype=np.float32)
    w1T = np.ascontiguousarray(np.asarray(w1, np.float32).T).astype(bf)
    w2 = np.asarray(w2, dtype=np.float32)
    b1 = np.asarray(b1, np.float32)
    b2 = np.asarray(b2, np.float32)
    lin_b = np.asarray(lin_b, np.float32)

    # centered/scaled fp8 operands for the big linear
    z8 = ((x - 0.5) * ZSCALE).astype(e4)               # [B, F] fp8
    w8 = (lin_w * WSCALE).astype(e4)                   # [O, F] fp8
    xb = x.astype(bf)                                  # [B, F] bf16 (FM se path)
    x28 = (x * x).astype(e4)                           # [B, F] fp8 (FM sq path)
    # exact rank-1 correction: linear = z@w + 0.5*colsum(w) + lin_b
    bias_c = PSC * (lin_b + 0.5 * lin_w.sum(axis=1, dtype=np.float64).astype(np.float32))

    zT_half = [np.ascontiguousarray(z8[g * Bc:(g + 1) * Bc, :].T) for g in range(2)]
    xbT_half = [np.ascontiguousarray(xb[g * Bc:(g + 1) * Bc, :].T) for g in range(2)]
    x2T_half = [np.ascontiguousarray(x28[g * Bc:(g + 1) * Bc, :].T) for g in range(2)]
    wT_q = [np.ascontiguousarray(w8[j * Oc:(j + 1) * Oc, :].T) for j in range(4)]
    w2T_q = [np.ascontiguousarray(w2[j * Oc:(j + 1) * Oc, :].T).astype(bf) for j in range(4)]
    in_maps = []
    for c in range(NCORES):
        g, j = c // 4, c % 4
        in_maps.append({
            "zT": zT_half[g],
            "xfm": np.ascontiguousarray(xbT_half[g][j * Fj:(j + 1) * Fj, :]),
            "x2fm": np.ascontiguousarray(x2T_half[g][j * Fj:(j + 1) * Fj, :]),
            "wT": wT_q[j],
            "embj": np.ascontiguousarray(emb[j * Fj:(j + 1) * Fj, :]).astype(bf),
            "embj2": np.ascontiguousarray(
                (emb[j * Fj:(j + 1) * Fj, :] ** 2) * E2SCALE).astype(e4),
            "w1T": w1T,
            "w2T": w2T_q[j],
            "b1": b1,
            "b2": np.ascontiguousarray(b2[j * Oc:(j + 1) * Oc]),
            "lb": np.ascontiguousarray(bias_c[j * Oc:(j + 1) * Oc]),
        })
    return in_maps


def _gather(results):
    """Assemble full outputs from per-core (g,j) blocks (upcast bf16)."""
    outs = {}
    for key in ("out_sum", "out_lin", "out_int"):
        full = np.empty((B, O), dtype=np.float32)
        for c in range(NCORES):
            g, j = c // 4, c % 4
            full[g * Bc:(g + 1) * Bc, j * Oc:(j + 1) * Oc] = \
                np.asarray(results[c][key]).astype(np.float32)
        outs[key] = full
    return outs["out_sum"], outs["out_lin"], outs["out_int"]


def kernel(sae_features, emb, lin_w, lin_b, w1, b1, w2, b2):
    from concourse.bass_utils import run_bass_kernel_spmd

    b2z = not np.any(np.asarray(b2))
    key = ("nc", b2z)
    if key not in _cache:
        _cache[key] = _build(b2_zero=b2z)
    nc = _cache[key]
    in_maps = _prep_inputs(sae_features, emb, lin_w, lin_b, w1, b1, w2, b2)
    try:
        res = run_bass_kernel_spmd(nc, in_maps, list(range(NCORES)))
    except Exception:
        # transient device desync/unrecoverable states heal on retry
        import time as _time
        _time.sleep(5)
        res = run_bass_kernel_spmd(nc, in_maps, list(range(NCORES)))
    return _gather(res.results)
